# revision 13
# baseline (speedup 1.0000x reference)
"""Causal self-attention (B=4, T=2048, C=1024, H=16) on 8 TRN2 NeuronCores.

Sharding: core = (batch, head-group) — data parallel over the 4 batches,
tensor parallel over 2 groups of 8 heads (Megatron-style column/row split of
the qkv / out projections).  Each core computes a [T, C] partial of the out
projection for its head group; the host sums the two partials per batch and
adds b_out, so no device collectives are needed.

v2: all matmuls in bf16 (tolerance 2e-2; measured ~4e-3).  Attention S for a
head pair lands in one 2-bank PSUM tile so a single 1024-wide ACTIVATE does
the exp for both heads (halves ScalarE instruction overhead).  Softmax
denominator rides along PV as a ones column in v; normalization uses
reciprocal_approx_fast + DRAM-bounce broadcast, writing normalized y^T
directly.  Projection (slab s+1) and out-projection (slab s-1) matmul units
are interleaved into the attention block loop as PE filler so the tensor
engine keeps streaming while ScalarE runs exp.
"""

import os
import sys
from contextlib import ExitStack

import numpy as np

for _p in ("/opt/trn_rl_repo", "/root/.axon_site/_ro/trn_rl_repo"):
    if os.path.isdir(_p) and _p not in sys.path:
        sys.path.append(_p)

import ml_dtypes

import concourse.bacc as bacc
import concourse.bass as bass
import concourse.tile as tile
from concourse import mybir
from concourse.bass_utils import run_bass_kernel_spmd
from concourse.masks import make_upper_triangular

AF = mybir.ActivationFunctionType
ALU = mybir.AluOpType
F32 = mybir.dt.float32
BF16 = mybir.dt.bfloat16

P = 128
SLAB = 512

B, T, C, H, D = 4, 2048, 1024, 16, 64
N_CORES = 8
N_GROUPS = 2          # head groups (tensor-parallel degree per batch)
HL = H // N_GROUPS    # heads per core
CL = HL * D           # local qkv width


def _build_nc(loop_reps=None):
    NCK = C // P          # 8  K-chunks of the projections
    MQK = 2 * CL // P     # 8  q+k output chunks
    MQ = MQK // 2         # 4
    TT = T // P           # 16
    NS = T // SLAB        # 4
    YC = CL // P          # 4
    W_OUT = 512
    NOUT = C // W_OUT     # 2
    scale = 1.0 / np.sqrt(D)

    nc = bacc.Bacc("TRN2", target_bir_lowering=False, debug=False,
                   num_devices=N_CORES)
    xT = nc.dram_tensor("xT", [C, T], BF16, kind="ExternalInput")
    wqk = nc.dram_tensor("wqk", [C, 2 * CL], BF16, kind="ExternalInput")
    wv = nc.dram_tensor("wv", [C, CL], BF16, kind="ExternalInput")
    wout = nc.dram_tensor("wout", [CL, C], BF16, kind="ExternalInput")
    bqk = nc.dram_tensor("bqk", [P, MQK], F32, kind="ExternalInput")
    bv = nc.dram_tensor("bv", [1, CL], F32, kind="ExternalInput")
    outp = nc.dram_tensor("outp", [T, C], F32, kind="ExternalOutput")
    scr = nc.dram_tensor("scr", [HL * NS, SLAB], F32)

    with tile.TileContext(nc) as tc, ExitStack() as ctx:
        pool = lambda name, bufs, **kw: ctx.enter_context(
            tc.tile_pool(name=name, bufs=bufs, **kw))

        const = pool("const", 1)
        kp = pool("kp", 1)
        vp = pool("vp", 1)
        wqkp = pool("wqkp", 1)
        wvp = pool("wvp", 1)
        woutp = pool("woutp", 1)
        xtp = pool("xt", 3)
        qp = pool("qp", 2)
        yTp = pool("yTp", 2)
        expp = pool("expp", 3)
        rp = pool("rp", 2)
        bip = pool("bip", 2)
        otp = pool("ot", 2)
        psP = pool("psP", 2, space="PSUM")    # proj + out-proj, 2 banks
        psS = pool("psS", 2, space="PSUM")    # S head-pair wide, 4 banks
        psY = pool("psY", 1, space="PSUM")    # PV accum, 2 banks

        k_sb = kp.tile([P, MQ, T], BF16)
        v_sb = vp.tile([P, TT, HL, D + 1], BF16)
        wqk_sb = wqkp.tile([P, NCK, 2 * CL], BF16)
        wv_sb = wvp.tile([P, NCK, CL], BF16)
        wout_sb = woutp.tile([P, YC, C], BF16)
        bqk_sb = const.tile([P, MQK], F32)
        bvb_sb = const.tile([P, CL], F32)
        mask01 = const.tile([P, P], BF16)
        maskf = const.tile([P, P], F32)
        onescr = const.tile([P, TT * HL], F32)

        nc.sync.dma_start(out=bqk_sb[:, :], in_=bqk[:, :])
        # broadcast v-bias row across 128 partitions straight from DRAM
        bv0 = bv[0:1, :]
        bv_bc = bass.AP(tensor=bv0.tensor, offset=bv0.offset,
                        ap=[[0, P], [1, CL]])
        nc.sync.dma_start(out=bvb_sb[:, :], in_=bv_bc)
        for c in range(NCK):
            nc.sync.dma_start(out=wqk_sb[:, c, :], in_=wqk[c * P:(c + 1) * P, :])
            nc.sync.dma_start(out=wv_sb[:, c, :], in_=wv[c * P:(c + 1) * P, :])
        for c in range(YC):
            nc.sync.dma_start(out=wout_sb[:, c, :], in_=wout[c * P:(c + 1) * P, :])
        # mask01[p, f] = 1 if f >= p else 0  (S^T visibility: tq >= tk).
        make_upper_triangular(nc, maskf[:, :], val=1.0, diag=True)
        nc.vector.tensor_copy(mask01[:, :], maskf[:, :])
        # ones column of v (bf16 can't be memset; copy from f32 scratch)
        nc.vector.memset(onescr[:, :], 1.0)
        nc.vector.tensor_copy(
            v_sb[:, :, :, D],
            onescr[:, :].rearrange("p (t h) -> p t h", h=HL))

        def emit_xt_dma(s, chunked=False):
            t0 = s * SLAB
            xt = xtp.tile([P, NCK, SLAB], BF16, tag="xt")
            if chunked:
                # per-chunk DMAs: first proj matmul only waits on chunk 0
                for c in range(NCK):
                    nc.sync.dma_start(
                        out=xt[:, c, :],
                        in_=xT[c * P:(c + 1) * P, t0:t0 + SLAB])
            else:
                # one 3-d DMA: dst [p, c, t] <- xT[c*P+p, t0+t]
                src0 = xT[0:P, t0:t0 + SLAB]
                src = bass.AP(tensor=src0.tensor, offset=src0.offset,
                              ap=[[T, P], [P * T, NCK], [1, SLAB]])
                nc.sync.dma_start(out=xt[:, :, :], in_=src)
            return xt

        def proj_units(s, xt):
            """A(s): qk projections (8 m-chunks) + v projection (4 subs),
            one PE unit each (8 accumulating matmuls + eviction)."""
            t0 = s * SLAB
            q_sb = qp.tile([P, MQ, SLAB], BF16, tag="q")

            def qk_unit(m):
                def emit():
                    ps = psP.tile([P, SLAB], F32, tag="ps", name="ps")
                    for c in range(NCK):
                        nc.tensor.matmul(
                            ps[:, :],
                            wqk_sb[:, c, m * P:(m + 1) * P],
                            xt[:, c, :],
                            start=(c == 0), stop=(c == NCK - 1))
                    dst = (q_sb[:, m, :] if m < MQ
                           else k_sb[:, m - MQ, t0:t0 + SLAB])
                    sc = scale if m < MQ else 1.0
                    nc.vector.tensor_scalar(
                        dst, ps[:, :], sc, bqk_sb[:, m:m + 1],
                        op0=ALU.mult, op1=ALU.add)
                return emit

            def v_unit(sub):
                def emit():
                    tt = s * (SLAB // P) + sub
                    ps = psP.tile([P, CL], F32, tag="ps", name="ps")
                    for c in range(NCK):
                        nc.tensor.matmul(
                            ps[:, :],
                            xt[:, c, sub * P:(sub + 1) * P],
                            wv_sb[:, c, :],
                            start=(c == 0), stop=(c == NCK - 1))
                    nc.vector.tensor_tensor(
                        v_sb[:, tt, :, 0:D],
                        ps[:, :].rearrange("p (h d) -> p h d", d=D),
                        bvb_sb[:, :].rearrange("p (h d) -> p h d", d=D),
                        op=ALU.add)
                return emit

            units = [qk_unit(m) for m in range(MQK)]
            vunits = [v_unit(sub) for sub in range(SLAB // P)]
            # order: q0,k0 then v (unblocks attention hp=0 asap), then rest
            units = ([units[0], units[MQ]] + vunits
                     + [units[m] for m in (1, MQ + 1, 2, MQ + 2, 3, MQ + 3)])
            return q_sb, units

        def out_units(s, yT_sb):
            """C(s): out projection, one PE unit per (sub, n chunk)."""
            t0 = s * SLAB

            ots = {}

            def o_unit(sub, n):
                def emit():
                    n0 = n * W_OUT
                    ps = psP.tile([P, W_OUT], F32, tag="ps", name="ps")
                    for c in range(YC):
                        nc.tensor.matmul(
                            ps[:, :],
                            yT_sb[c][:, sub * P:(sub + 1) * P],
                            wout_sb[:, c, n0:n0 + W_OUT],
                            start=(c == 0), stop=(c == YC - 1))
                    if n == 0:
                        ots[sub] = otp.tile([P, C], F32, tag="ot",
                                            name="ot")
                    ot = ots[sub]
                    nc.vector.tensor_copy(ot[:, n0:n0 + W_OUT], ps[:, :])
                    if n == NOUT - 1:
                        nc.sync.dma_start(
                            out=outp[t0 + sub * P:t0 + (sub + 1) * P, :],
                            in_=ot[:, :])
                return emit

            return [o_unit(sub, n)
                    for sub in range(SLAB // P) for n in range(NOUT)]

        def body():
            fillers = []

            def pop_filler():
                if fillers:
                    fillers.pop(0)()

            xt = emit_xt_dma(0, chunked=True)
            q_sb, units0 = proj_units(0, xt)
            for u in units0:   # A(0) prologue, nothing to overlap with
                u()
            yT_prev = None

            for s in range(NS):
                t0 = s * SLAB
                nblk = (s + 1) * SLAB // P
                # stage fillers: A(s+1) then C(s-1)
                fillers = []
                if s + 1 < NS:
                    xt = emit_xt_dma(s + 1)
                    q_next, units = proj_units(s + 1, xt)
                else:
                    q_next, units = None, []
                fillers += units
                if yT_prev is not None:
                    fillers += out_units(s - 1, yT_prev)
                nfill = len(fillers)
                tot_blocks = 4 * nblk
                # pacing: spread fillers across the block loop
                fill_every = max(1, tot_blocks // max(1, nfill))

                yT_sb = []
                for c in range(YC):
                    yTc = yTp.tile([P, SLAB], BF16, tag=f"yT{c}",
                                   name=f"yT{c}")
                    yT_sb.append(yTc)
                blk_i = 0
                for hp in range(HL // 2):
                    py0 = psY.tile([D + 1, SLAB], F32, tag="py0", name="py0")
                    py1 = psY.tile([D + 1, SLAB], F32, tag="py1", name="py1")
                    pys = (py0, py1)

                    def emit_s(b):
                        # S-pair matmuls + visible-region exp + diag mask
                        tk0 = b * P
                        off = tk0 - t0
                        vis = max(0, off)
                        ps = psS.tile([P, 2, SLAB], F32, tag="s", name="s")
                        for i in range(2):
                            row0 = i * 64
                            nc.tensor.matmul(
                                ps[:, i, vis:SLAB],
                                k_sb[row0:row0 + 64, hp, tk0:tk0 + P],
                                q_sb[row0:row0 + 64, hp, vis:SLAB],
                                start=True, stop=True,
                                tile_position=(row0, 0))
                        ep = expp.tile([P, 2, SLAB], BF16, tag="ep")
                        nc.scalar.activation(ep[:, :, vis:SLAB],
                                             ps[:, :, vis:SLAB], AF.Exp)
                        if off >= 0:
                            for i in range(2):
                                nc.gpsimd.tensor_mul(
                                    ep[:, i, off:off + P],
                                    ep[:, i, off:off + P], mask01[:, :])
                        return ep, vis

                    ep_b, vis_b = emit_s(0)
                    for b in range(nblk):
                        # prefetch next block's S while exp(b) runs
                        if b + 1 < nblk:
                            ep_n, vis_n = emit_s(b + 1)
                        blk_i += 1
                        if blk_i % fill_every == 0:
                            pop_filler()
                        for i in range(2):
                            nc.tensor.matmul(
                                pys[i][0:D + 1, vis_b:SLAB],
                                v_sb[:, b, 2 * hp + i, 0:D + 1],
                                ep_b[:, i, vis_b:SLAB],
                                start=(b == 0), stop=(b == nblk - 1))
                        if b + 1 < nblk:
                            ep_b, vis_b = ep_n, vis_n
                    # stage PV out of PSUM fast (frees psY for next hp),
                    # then normalize from SBUF: broadcast the denominator
                    # row via DRAM bounce, reciprocal on the wide tile
                    sts = []
                    for i in range(2):
                        st = rp.tile([D + 1, SLAB], F32, tag="st")
                        nc.vector.tensor_copy(st[:, :], pys[i][:, :])
                        sts.append(st)
                    for i in range(2):
                        h = 2 * hp + i
                        row0 = i * 64
                        sidx = h * NS + s
                        nc.sync.dma_start(out=scr[sidx:sidx + 1, :],
                                          in_=sts[i][D:D + 1, :])
                        src = scr[sidx:sidx + 1, :]
                        bsrc = bass.AP(tensor=src.tensor, offset=src.offset,
                                       ap=[[0, 64], [1, SLAB]])
                        bi = bip.tile([64, SLAB], F32, tag="bi")
                        biR = bip.tile([64, SLAB], F32, tag="biR")
                        nc.sync.dma_start(out=bi[:, :], in_=bsrc)
                        nc.vector.reciprocal_approx_fast(biR[:, :], bi[:, :])
                        nc.vector.tensor_tensor(
                            yT_sb[hp][row0:row0 + 64, :],
                            sts[i][0:D, :], biR[:, :], op=ALU.mult)
                # drain any remaining fillers for this slab
                while fillers:
                    pop_filler()
                q_sb = q_next
                yT_prev = yT_sb

            # C(NS-2) was interleaved; C(NS-1) epilogue
            for u in out_units(NS - 1, yT_prev):
                u()

        if loop_reps is None:
            body()
        else:
            with tc.For_i(0, loop_reps, 1):
                body()

    nc.compile()
    return nc


_NC_CACHE = None


def _get_nc():
    global _NC_CACHE
    if _NC_CACHE is None:
        _NC_CACHE = _build_nc()
    return _NC_CACHE


def _bf16(a):
    return np.ascontiguousarray(a.astype(ml_dtypes.bfloat16))


def make_in_maps(x, W_qkv, b_qkv, W_out):
    scale = 1.0 / np.sqrt(D)
    MQK = 2 * CL // P
    in_maps = []
    for core in range(N_CORES):
        b, hg = divmod(core, N_GROUPS)
        qs = slice(hg * CL, (hg + 1) * CL)
        ks = slice(C + hg * CL, C + (hg + 1) * CL)
        vs = slice(2 * C + hg * CL, 2 * C + (hg + 1) * CL)
        bqk_cat = np.concatenate([b_qkv[qs] * scale, b_qkv[ks]])
        in_maps.append({
            "xT": _bf16(x[b].T),
            "wqk": _bf16(np.concatenate([W_qkv[:, qs], W_qkv[:, ks]], axis=1)),
            "wv": _bf16(W_qkv[:, vs]),
            "wout": _bf16(W_out[hg * CL:(hg + 1) * CL, :]),
            "bqk": np.ascontiguousarray(bqk_cat.reshape(MQK, P).T),
            "bv": np.ascontiguousarray(
                b_qkv[vs].reshape(1, CL).astype(np.float32)),
        })
    return in_maps


def kernel(x, W_qkv, b_qkv, W_out, b_out):
    x = np.asarray(x, dtype=np.float32)
    W_qkv = np.asarray(W_qkv, dtype=np.float32)
    b_qkv = np.asarray(b_qkv, dtype=np.float32)
    W_out = np.asarray(W_out, dtype=np.float32)
    b_out = np.asarray(b_out, dtype=np.float32)

    nc = _get_nc()
    in_maps = make_in_maps(x, W_qkv, b_qkv, W_out)
    res = run_bass_kernel_spmd(nc, in_maps, core_ids=list(range(N_CORES)))

    out = np.empty((B, T, C), dtype=np.float32)
    for b in range(B):
        out[b] = (res.results[N_GROUPS * b]["outp"]
                  + res.results[N_GROUPS * b + 1]["outp"] + b_out)
    return out


# revision 15
# speedup vs baseline: 1.0028x; 1.0028x over previous
"""Causal self-attention (B=4, T=2048, C=1024, H=16) on 8 TRN2 NeuronCores.

Sharding: core = (batch, head-group) — data parallel over the 4 batches,
tensor parallel over 2 groups of 8 heads (Megatron-style column/row split of
the qkv / out projections).  Each core computes a [T, C] partial of the out
projection for its head group; the host sums the two partials per batch and
adds b_out, so no device collectives are needed.

v2: all matmuls in bf16 (tolerance 2e-2; measured ~4e-3).  Attention S for a
head pair lands in one 2-bank PSUM tile so a single 1024-wide ACTIVATE does
the exp for both heads (halves ScalarE instruction overhead).  Softmax
denominator rides along PV as a ones column in v; normalization uses
reciprocal_approx_fast + DRAM-bounce broadcast, writing normalized y^T
directly.  Projection (slab s+1) and out-projection (slab s-1) matmul units
are interleaved into the attention block loop as PE filler so the tensor
engine keeps streaming while ScalarE runs exp.
"""

import os
import sys
from contextlib import ExitStack

import numpy as np

for _p in ("/opt/trn_rl_repo", "/root/.axon_site/_ro/trn_rl_repo"):
    if os.path.isdir(_p) and _p not in sys.path:
        sys.path.append(_p)

import ml_dtypes

import concourse.bacc as bacc
import concourse.bass as bass
import concourse.tile as tile
from concourse import mybir
from concourse.bass_utils import run_bass_kernel_spmd
from concourse.masks import make_upper_triangular

AF = mybir.ActivationFunctionType
ALU = mybir.AluOpType
F32 = mybir.dt.float32
BF16 = mybir.dt.bfloat16

P = 128
SLAB = 512

B, T, C, H, D = 4, 2048, 1024, 16, 64
N_CORES = 8
N_GROUPS = 2          # head groups (tensor-parallel degree per batch)
HL = H // N_GROUPS    # heads per core
CL = HL * D           # local qkv width


def _build_nc(loop_reps=None):
    NCK = C // P          # 8  K-chunks of the projections
    MQK = 2 * CL // P     # 8  q+k output chunks
    MQ = MQK // 2         # 4
    TT = T // P           # 16
    NS = T // SLAB        # 4
    YC = CL // P          # 4
    W_OUT = 512
    NOUT = C // W_OUT     # 2
    scale = 1.0 / np.sqrt(D)

    nc = bacc.Bacc("TRN2", target_bir_lowering=False, debug=False,
                   num_devices=N_CORES)
    xT = nc.dram_tensor("xT", [C, T], BF16, kind="ExternalInput")
    wqk = nc.dram_tensor("wqk", [C, 2 * CL], BF16, kind="ExternalInput")
    wv = nc.dram_tensor("wv", [C, CL], BF16, kind="ExternalInput")
    wout = nc.dram_tensor("wout", [CL, C], BF16, kind="ExternalInput")
    bqk = nc.dram_tensor("bqk", [P, MQK], F32, kind="ExternalInput")
    bv = nc.dram_tensor("bv", [1, CL], F32, kind="ExternalInput")
    outp = nc.dram_tensor("outp", [T, C], F32, kind="ExternalOutput")
    scr = nc.dram_tensor("scr", [HL * NS, SLAB], F32)

    with tile.TileContext(nc) as tc, ExitStack() as ctx:
        pool = lambda name, bufs, **kw: ctx.enter_context(
            tc.tile_pool(name=name, bufs=bufs, **kw))

        const = pool("const", 1)
        kp = pool("kp", 1)
        vp = pool("vp", 1)
        wqkp = pool("wqkp", 1)
        wvp = pool("wvp", 1)
        woutp = pool("woutp", 1)
        xtp = pool("xt", 3)
        qp = pool("qp", 2)
        yTp = pool("yTp", 2)
        expp = pool("expp", 3)
        rp = pool("rp", 2)
        bip = pool("bip", 2)
        otp = pool("ot", 2)
        psP = pool("psP", 2, space="PSUM")    # proj + out-proj, 2 banks
        psS = pool("psS", 2, space="PSUM")    # S head-pair wide, 4 banks
        psY = pool("psY", 1, space="PSUM")    # PV accum, 2 banks

        k_sb = kp.tile([P, MQ, T], BF16)
        v_sb = vp.tile([P, TT, HL, D + 1], BF16)
        wqk_sb = wqkp.tile([P, NCK, 2 * CL], BF16)
        wv_sb = wvp.tile([P, NCK, CL], BF16)
        wout_sb = woutp.tile([P, YC, C], BF16)
        bqk_sb = const.tile([P, MQK], F32)
        bvb_sb = const.tile([P, CL], F32)
        mask01 = const.tile([P, P], BF16)
        maskf = const.tile([P, P], F32)
        onescr = const.tile([P, TT * HL], F32)

        nc.sync.dma_start(out=bqk_sb[:, :], in_=bqk[:, :])
        # broadcast v-bias row across 128 partitions straight from DRAM
        bv0 = bv[0:1, :]
        bv_bc = bass.AP(tensor=bv0.tensor, offset=bv0.offset,
                        ap=[[0, P], [1, CL]])
        nc.sync.dma_start(out=bvb_sb[:, :], in_=bv_bc)
        for c in range(NCK):
            nc.sync.dma_start(out=wqk_sb[:, c, :], in_=wqk[c * P:(c + 1) * P, :])
            nc.sync.dma_start(out=wv_sb[:, c, :], in_=wv[c * P:(c + 1) * P, :])
        for c in range(YC):
            nc.sync.dma_start(out=wout_sb[:, c, :], in_=wout[c * P:(c + 1) * P, :])
        # mask01[p, f] = 1 if f >= p else 0  (S^T visibility: tq >= tk).
        make_upper_triangular(nc, maskf[:, :], val=1.0, diag=True)
        nc.vector.tensor_copy(mask01[:, :], maskf[:, :])
        # ones column of v (bf16 can't be memset; copy from f32 scratch)
        nc.vector.memset(onescr[:, :], 1.0)
        nc.vector.tensor_copy(
            v_sb[:, :, :, D],
            onescr[:, :].rearrange("p (t h) -> p t h", h=HL))

        def emit_xt_dma(s, chunked=False):
            t0 = s * SLAB
            xt = xtp.tile([P, NCK, SLAB], BF16, tag="xt")
            if chunked:
                # per-chunk DMAs: first proj matmul only waits on chunk 0
                for c in range(NCK):
                    nc.sync.dma_start(
                        out=xt[:, c, :],
                        in_=xT[c * P:(c + 1) * P, t0:t0 + SLAB])
            else:
                # one 3-d DMA: dst [p, c, t] <- xT[c*P+p, t0+t]
                src0 = xT[0:P, t0:t0 + SLAB]
                src = bass.AP(tensor=src0.tensor, offset=src0.offset,
                              ap=[[T, P], [P * T, NCK], [1, SLAB]])
                nc.sync.dma_start(out=xt[:, :, :], in_=src)
            return xt

        def proj_units(s, xt):
            """A(s): qk projections (8 m-chunks) + v projection (4 subs),
            one PE unit each (8 accumulating matmuls + eviction)."""
            t0 = s * SLAB
            q_sb = qp.tile([P, MQ, SLAB], BF16, tag="q")

            def qk_unit(m):
                def emit():
                    ps = psP.tile([P, SLAB], F32, tag="ps", name="ps")
                    for c in range(NCK):
                        nc.tensor.matmul(
                            ps[:, :],
                            wqk_sb[:, c, m * P:(m + 1) * P],
                            xt[:, c, :],
                            start=(c == 0), stop=(c == NCK - 1))
                    dst = (q_sb[:, m, :] if m < MQ
                           else k_sb[:, m - MQ, t0:t0 + SLAB])
                    sc = scale if m < MQ else 1.0
                    nc.vector.tensor_scalar(
                        dst, ps[:, :], sc, bqk_sb[:, m:m + 1],
                        op0=ALU.mult, op1=ALU.add)
                return emit

            def v_unit(sub):
                def emit():
                    tt = s * (SLAB // P) + sub
                    ps = psP.tile([P, CL], F32, tag="ps", name="ps")
                    for c in range(NCK):
                        nc.tensor.matmul(
                            ps[:, :],
                            xt[:, c, sub * P:(sub + 1) * P],
                            wv_sb[:, c, :],
                            start=(c == 0), stop=(c == NCK - 1))
                    nc.vector.tensor_tensor(
                        v_sb[:, tt, :, 0:D],
                        ps[:, :].rearrange("p (h d) -> p h d", d=D),
                        bvb_sb[:, :].rearrange("p (h d) -> p h d", d=D),
                        op=ALU.add)
                return emit

            units = [qk_unit(m) for m in range(MQK)]
            vunits = [v_unit(sub) for sub in range(SLAB // P)]
            # order: q0,k0 then v (unblocks attention hp=0 asap), then rest
            units = ([units[0], units[MQ]] + vunits
                     + [units[m] for m in (1, MQ + 1, 2, MQ + 2, 3, MQ + 3)])
            return q_sb, units

        def out_units(s, yT_sb):
            """C(s): out projection, one PE unit per (sub, n chunk)."""
            t0 = s * SLAB

            ots = {}

            def o_unit(sub, n):
                def emit():
                    n0 = n * W_OUT
                    ps = psP.tile([P, W_OUT], F32, tag="ps", name="ps")
                    for c in range(YC):
                        nc.tensor.matmul(
                            ps[:, :],
                            yT_sb[c][:, sub * P:(sub + 1) * P],
                            wout_sb[:, c, n0:n0 + W_OUT],
                            start=(c == 0), stop=(c == YC - 1))
                    if n == 0:
                        ots[sub] = otp.tile([P, C], F32, tag="ot",
                                            name="ot")
                    ot = ots[sub]
                    nc.vector.tensor_copy(ot[:, n0:n0 + W_OUT], ps[:, :])
                    if n == NOUT - 1:
                        nc.sync.dma_start(
                            out=outp[t0 + sub * P:t0 + (sub + 1) * P, :],
                            in_=ot[:, :])
                return emit

            return [o_unit(sub, n)
                    for sub in range(SLAB // P) for n in range(NOUT)]

        def body():
            fillers = []

            def pop_filler():
                if fillers:
                    fillers.pop(0)()

            xt = emit_xt_dma(0, chunked=True)
            q_sb, units0 = proj_units(0, xt)
            for u in units0:   # A(0) prologue, nothing to overlap with
                u()
            yT_prev = None

            for s in range(NS):
                t0 = s * SLAB
                nblk = (s + 1) * SLAB // P
                # stage fillers: A(s+1) then C(s-1)
                fillers = []
                if s + 1 < NS:
                    xt = emit_xt_dma(s + 1)
                    q_next, units = proj_units(s + 1, xt)
                else:
                    q_next, units = None, []
                fillers += units
                if yT_prev is not None:
                    fillers += out_units(s - 1, yT_prev)
                nfill = len(fillers)
                tot_blocks = 4 * nblk
                # pacing: spread fillers across the block loop
                fill_every = max(1, tot_blocks // max(1, nfill))

                yT_sb = []
                for c in range(YC):
                    yTc = yTp.tile([P, SLAB], BF16, tag=f"yT{c}",
                                   name=f"yT{c}")
                    yT_sb.append(yTc)
                blk_i = 0
                for hp in range(HL // 2):
                    py0 = psY.tile([D + 1, SLAB], F32, tag="py0", name="py0")
                    py1 = psY.tile([D + 1, SLAB], F32, tag="py1", name="py1")
                    pys = (py0, py1)

                    def emit_s(b):
                        # S-pair matmuls + visible-region exp + diag mask
                        tk0 = b * P
                        off = tk0 - t0
                        vis = max(0, off)
                        ps = psS.tile([P, 2, SLAB], F32, tag="s", name="s")
                        for i in range(2):
                            row0 = i * 64
                            nc.tensor.matmul(
                                ps[:, i, vis:SLAB],
                                k_sb[row0:row0 + 64, hp, tk0:tk0 + P],
                                q_sb[row0:row0 + 64, hp, vis:SLAB],
                                start=True, stop=True,
                                tile_position=(row0, 0))
                        ep = expp.tile([P, 2, SLAB], BF16, tag="ep")
                        nc.scalar.activation(ep[:, :, vis:SLAB],
                                             ps[:, :, vis:SLAB], AF.Exp)
                        if off >= 0:
                            for i in range(2):
                                nc.gpsimd.tensor_mul(
                                    ep[:, i, off:off + P],
                                    ep[:, i, off:off + P], mask01[:, :])
                        return ep, vis

                    ep_b, vis_b = emit_s(0)
                    for b in range(nblk):
                        # prefetch next block's S while exp(b) runs
                        if b + 1 < nblk:
                            ep_n, vis_n = emit_s(b + 1)
                        blk_i += 1
                        if blk_i % fill_every == 0:
                            pop_filler()
                        for i in range(2):
                            nc.tensor.matmul(
                                pys[i][0:D + 1, vis_b:SLAB],
                                v_sb[:, b, 2 * hp + i, 0:D + 1],
                                ep_b[:, i, vis_b:SLAB],
                                start=(b == 0), stop=(b == nblk - 1))
                        if b + 1 < nblk:
                            ep_b, vis_b = ep_n, vis_n
                    # stage PV out of PSUM fast (frees psY for next hp),
                    # then normalize from SBUF: broadcast the denominator
                    # row via DRAM bounce, reciprocal on the wide tile
                    sts = []
                    for i in range(2):
                        st = rp.tile([D + 1, SLAB], F32, tag="st")
                        nc.vector.tensor_copy(st[:, :], pys[i][:, :])
                        sts.append(st)
                        sidx = (2 * hp + i) * NS + s
                        nc.sync.dma_start(out=scr[sidx:sidx + 1, :],
                                          in_=st[D:D + 1, :])
                    for i in range(2):
                        h = 2 * hp + i
                        row0 = i * 64
                        sidx = h * NS + s
                        src = scr[sidx:sidx + 1, :]
                        bsrc = bass.AP(tensor=src.tensor, offset=src.offset,
                                       ap=[[0, 64], [1, SLAB]])
                        bi = bip.tile([64, SLAB], F32, tag="bi")
                        biR = bip.tile([64, SLAB], F32, tag="biR")
                        nc.sync.dma_start(out=bi[:, :], in_=bsrc)
                        nc.vector.reciprocal_approx_fast(biR[:, :], bi[:, :])
                        nc.vector.tensor_tensor(
                            yT_sb[hp][row0:row0 + 64, :],
                            sts[i][0:D, :], biR[:, :], op=ALU.mult)
                # drain any remaining fillers for this slab
                while fillers:
                    pop_filler()
                q_sb = q_next
                yT_prev = yT_sb

            # C(NS-2) was interleaved; C(NS-1) epilogue
            for u in out_units(NS - 1, yT_prev):
                u()

        if loop_reps is None:
            body()
        else:
            with tc.For_i(0, loop_reps, 1):
                body()

    nc.compile()
    return nc


_NC_CACHE = None


def _get_nc():
    global _NC_CACHE
    if _NC_CACHE is None:
        _NC_CACHE = _build_nc()
    return _NC_CACHE


def _bf16(a):
    return np.ascontiguousarray(a.astype(ml_dtypes.bfloat16))


def make_in_maps(x, W_qkv, b_qkv, W_out):
    scale = 1.0 / np.sqrt(D)
    MQK = 2 * CL // P
    in_maps = []
    for core in range(N_CORES):
        b, hg = divmod(core, N_GROUPS)
        qs = slice(hg * CL, (hg + 1) * CL)
        ks = slice(C + hg * CL, C + (hg + 1) * CL)
        vs = slice(2 * C + hg * CL, 2 * C + (hg + 1) * CL)
        bqk_cat = np.concatenate([b_qkv[qs] * scale, b_qkv[ks]])
        in_maps.append({
            "xT": _bf16(x[b].T),
            "wqk": _bf16(np.concatenate([W_qkv[:, qs], W_qkv[:, ks]], axis=1)),
            "wv": _bf16(W_qkv[:, vs]),
            "wout": _bf16(W_out[hg * CL:(hg + 1) * CL, :]),
            "bqk": np.ascontiguousarray(bqk_cat.reshape(MQK, P).T),
            "bv": np.ascontiguousarray(
                b_qkv[vs].reshape(1, CL).astype(np.float32)),
        })
    return in_maps


def kernel(x, W_qkv, b_qkv, W_out, b_out):
    x = np.asarray(x, dtype=np.float32)
    W_qkv = np.asarray(W_qkv, dtype=np.float32)
    b_qkv = np.asarray(b_qkv, dtype=np.float32)
    W_out = np.asarray(W_out, dtype=np.float32)
    b_out = np.asarray(b_out, dtype=np.float32)

    nc = _get_nc()
    in_maps = make_in_maps(x, W_qkv, b_qkv, W_out)
    res = run_bass_kernel_spmd(nc, in_maps, core_ids=list(range(N_CORES)))

    out = np.empty((B, T, C), dtype=np.float32)
    for b in range(B):
        out[b] = (res.results[N_GROUPS * b]["outp"]
                  + res.results[N_GROUPS * b + 1]["outp"] + b_out)
    return out
